# revision 7
# baseline (speedup 1.0000x reference)
"""Trainium2 Bass kernel for nn_Dec_9285719294079 (LSTM decoder, teacher forcing).

Model (per reference):
    g = emb[gtruths].transpose(1,0,2)              # [T, B, D]
    per step: gates = [enc, g_t] @ W_ih.T + b_ih + h @ W_hh.T + b_hh
              i,f,gg,o = split(gates); c = s(f)*c + s(i)*tanh(gg); h = s(o)*tanh(c)
              logits_t = h @ W_fc.T + b_fc         # [B, V]
    outputs: logits [T, B, V], predic = argmax(logits, -1).T  [B, T]

Distribution over 8 NeuronCores:
  - Recurrence gate-sharded: core k owns 128 rows of each of the i/f/o/g gate
    blocks (512 gate rows). Per step, the core computes its gate slice,
    produces its h slice [B=32, 128], transposes it, splits it into a bf16
    hi/lo pair, and the 8 cores AllGather h^T (16KB) so everyone has the full
    h for the next step.
  - The input-side gates Xg = [enc, g_t] @ W_ih_k.T are precomputed for all
    T*B rows before the recurrence starts.
  - Logits vocab-sharded: core k owns vocab slice [4000]. h^T tiles from the
    AllGather feed [128-row, 4000] logits matmuls, batched 4 timesteps at a
    time, interleaved into the recurrence's idle PE time.
  - All matmuls use bf16 hi/lo 3-term products (hi*hi + hi*lo + lo*hi with
    fp32 PSUM accumulation): ~2^-17 relative precision at full PE rate.
  - argmax on device via vector max / max_index (first-index tie semantics,
    matching jnp.argmax); host combines the 8 per-core (value, index) pairs.
"""

import sys
import os

for _p in ("/opt/trn_rl_repo", "/root/.axon_site/_ro/trn_rl_repo"):
    if os.path.isdir(_p) and _p not in sys.path:
        sys.path.insert(0, _p)

import numpy as np
import ml_dtypes

import concourse.bass as bass
import concourse.mybir as mybir
import concourse.tile as tile
from concourse import bacc
from concourse import bass_utils
from concourse.masks import make_identity

BF16 = ml_dtypes.bfloat16
NC = 8
B, T, V, D, E, H = 32, 64, 32000, 512, 1024, 1024
GS = 4 * H // NC          # gate rows per core (512)
VS = V // NC              # vocab slice per core (4000)
MT = T * B // 128         # logits m-tiles (16)
KH = H // 128             # k-tiles over H (8)
KE = E // 128             # k-tiles over E (8)
KD = D // 128             # k-tiles over D (4)
NV = VS // 500            # logits n-tiles (8)

_PROG_CACHE = {}


def _build_program(gate_bias: bool, fc_bias: bool):
    f32 = mybir.dt.float32
    bf = mybir.dt.bfloat16
    u32 = mybir.dt.uint32
    AF = mybir.ActivationFunctionType

    nc = bacc.Bacc("TRN2", target_bir_lowering=False, debug=False,
                   num_devices=NC)

    # ---------------- external inputs (per core, already sharded/split) ----
    def din(name, shape):
        return nc.dram_tensor(name, shape, bf, kind="ExternalInput")

    encT_hi = din("encT_hi", [E, B]);      encT_lo = din("encT_lo", [E, B])
    GT_hi = din("GT_hi", [D, T * B]);      GT_lo = din("GT_lo", [D, T * B])
    wie_hi = din("wie_hi", [E, GS]);       wie_lo = din("wie_lo", [E, GS])
    wid_hi = din("wid_hi", [D, GS]);       wid_lo = din("wid_lo", [D, GS])
    whh_hi_d = din("whh_hi", [H, GS]);     whh_lo_d = din("whh_lo", [H, GS])
    wfc_hi_d = din("wfc_hi", [H, VS]);     wfc_lo_d = din("wfc_lo", [H, VS])
    if gate_bias:
        bg_hi = din("bg_hi", [1, GS]);     bg_lo = din("bg_lo", [1, GS])
    if fc_bias:
        bf_hi = din("bf_hi", [1, VS]);     bf_lo = din("bf_lo", [1, VS])

    logits_o = nc.dram_tensor("logits_o", [T * B, VS], f32, kind="ExternalOutput")
    maxv_o = nc.dram_tensor("maxv_o", [T * B, 1], f32, kind="ExternalOutput")
    maxi_o = nc.dram_tensor("maxi_o", [T * B, 1], f32, kind="ExternalOutput")

    with tile.TileContext(nc) as tc:
        # ------------- persistent pools -------------
        cpool = tc.alloc_tile_pool(name="consts", bufs=1)
        whh_pool = tc.alloc_tile_pool(name="whh", bufs=1)
        xg_pool = tc.alloc_tile_pool(name="xg", bufs=4)
        stage_pool = tc.alloc_tile_pool(name="stage", bufs=3)
        small_pool = tc.alloc_tile_pool(name="small", bufs=2)
        agp_pool = tc.alloc_tile_pool(name="agp", bufs=2)
        amax_pool = tc.alloc_tile_pool(name="amax", bufs=2)
        gps_pool = tc.alloc_tile_pool(name="gps", bufs=2, space="PSUM")
        trp_pool = tc.alloc_tile_pool(name="trp", bufs=2, space="PSUM")
        dram_pool = tc.alloc_tile_pool(name="dram", bufs=1, space="DRAM")
        agd_pool = tc.alloc_tile_pool(name="agd", bufs=2, space="DRAM")

        # constants
        ident32f = cpool.tile([32, 32], f32)
        make_identity(nc, ident32f)
        ident32b = cpool.tile([32, 32], bf)
        make_identity(nc, ident32b)
        i32x4 = cpool.tile([32, 128], bf)
        nc.gpsimd.memset(i32x4, 0.0)
        for j in range(4):
            make_identity(nc, i32x4[:, 32 * j:32 * (j + 1)], nomemset=True)
        if gate_bias or fc_bias:
            ones1 = cpool.tile([1, 128], bf)
            nc.gpsimd.memset(ones1, 1.0)

        # recurrence weights (resident)
        whh_hi = whh_pool.tile([128, KH, GS], bf)
        whh_lo = whh_pool.tile([128, KH, GS], bf)
        nc.sync.dma_start(out=whh_hi, in_=whh_hi_d.ap().rearrange("(k p) g -> p k g", p=128))
        nc.sync.dma_start(out=whh_lo, in_=whh_lo_d.ap().rearrange("(k p) g -> p k g", p=128))
        if gate_bias:
            bg_hi_sb = cpool.tile([1, GS], bf)
            bg_lo_sb = cpool.tile([1, GS], bf)
            nc.sync.dma_start(out=bg_hi_sb, in_=bg_hi[:])
            nc.sync.dma_start(out=bg_lo_sb, in_=bg_lo[:])
        if fc_bias:
            bf_hi_sb = cpool.tile([1, VS], bf)
            bf_lo_sb = cpool.tile([1, VS], bf)
            nc.sync.dma_start(out=bf_hi_sb, in_=bf_hi[:])
            nc.sync.dma_start(out=bf_lo_sb, in_=bf_lo[:])

        # Xg staging in DRAM: [T*B, 1024] bf16, cols 0:512 hi, 512:1024 lo
        xg_dram = dram_pool.tile([T * B, 2 * GS], bf)

        # --------------- phase A: Xg = [enc, g] @ W_ih_k.T (+ gate bias) ----
        with tc.tile_pool(name="phaseA", bufs=1) as pa, \
             tc.tile_pool(name="phaseA_ps", bufs=2, space="PSUM") as pa_ps:
            enc_hi = pa.tile([128, KE, B], bf)
            enc_lo = pa.tile([128, KE, B], bf)
            nc.sync.dma_start(out=enc_hi, in_=encT_hi.ap().rearrange("(k p) b -> p k b", p=128))
            nc.sync.dma_start(out=enc_lo, in_=encT_lo.ap().rearrange("(k p) b -> p k b", p=128))
            wieh = pa.tile([128, KE, GS], bf)
            wiel = pa.tile([128, KE, GS], bf)
            nc.sync.dma_start(out=wieh, in_=wie_hi.ap().rearrange("(k p) g -> p k g", p=128))
            nc.sync.dma_start(out=wiel, in_=wie_lo.ap().rearrange("(k p) g -> p k g", p=128))
            widh = pa.tile([128, KD, GS], bf)
            widl = pa.tile([128, KD, GS], bf)
            nc.sync.dma_start(out=widh, in_=wid_hi.ap().rearrange("(k p) g -> p k g", p=128))
            nc.sync.dma_start(out=widl, in_=wid_lo.ap().rearrange("(k p) g -> p k g", p=128))
            gth = pa.tile([128, KD, T * B], bf)
            gtl = pa.tile([128, KD, T * B], bf)
            nc.sync.dma_start(out=gth, in_=GT_hi.ap().rearrange("(k p) m -> p k m", p=128))
            nc.sync.dma_start(out=gtl, in_=GT_lo.ap().rearrange("(k p) m -> p k m", p=128))

            # encoder part: encg [B, GS] (+ bias folded here)
            encg_ps = pa_ps.tile([B, GS], f32, bufs=1)
            mms = []
            for k in range(KE):
                mms.append((enc_hi[:, k, :], wieh[:, k, :]))
                mms.append((enc_hi[:, k, :], wiel[:, k, :]))
                mms.append((enc_lo[:, k, :], wieh[:, k, :]))
            if gate_bias:
                mms.append((ones1[:, 0:B], bg_hi_sb[:]))
                mms.append((ones1[:, 0:B], bg_lo_sb[:]))
            for i, (lt, rh) in enumerate(mms):
                nc.tensor.matmul(encg_ps, lhsT=lt, rhs=rh,
                                 start=(i == 0), stop=(i == len(mms) - 1))
            encg = pa.tile([B, 2 * GS], bf)
            nc.scalar.copy(encg[:, 0:GS], encg_ps)
            nc.vector.tensor_sub(out=encg[:, GS:2 * GS], in0=encg_ps,
                                 in1=encg[:, 0:GS])

            # G part per m-tile, + encg broadcast via block-identity
            for m in range(MT):
                xps = pa_ps.tile([128, GS], f32, name=f"xps{m}", tag="xps")
                mms = []
                for k in range(KD):
                    mms.append((gth[:, k, m * 128:(m + 1) * 128], widh[:, k, :]))
                    mms.append((gth[:, k, m * 128:(m + 1) * 128], widl[:, k, :]))
                    mms.append((gtl[:, k, m * 128:(m + 1) * 128], widh[:, k, :]))
                mms.append((i32x4[:], encg[:, 0:GS]))
                mms.append((i32x4[:], encg[:, GS:2 * GS]))
                for i, (lt, rh) in enumerate(mms):
                    nc.tensor.matmul(xps, lhsT=lt, rhs=rh,
                                     start=(i == 0), stop=(i == len(mms) - 1))
                xpk = pa.tile([128, 2 * GS], bf, name=f"xpk{m}", tag="xpk",
                              bufs=2)
                nc.scalar.copy(xpk[:, 0:GS], xps)
                nc.vector.tensor_sub(out=xpk[:, GS:2 * GS], in0=xps,
                                     in1=xpk[:, 0:GS])
                nc.sync.dma_start(out=xg_dram[m * 128:(m + 1) * 128, :], in_=xpk)

        # vocab projection weights (loaded after phase A frees SBUF)
        wfc_pool = tc.alloc_tile_pool(name="wfc", bufs=1)
        lsb_pool = tc.alloc_tile_pool(name="lsb", bufs=2)
        lps_pool = tc.alloc_tile_pool(name="lps", bufs=3, space="PSUM")
        wfc_hi = wfc_pool.tile([128, KH, VS], bf)
        wfc_lo = wfc_pool.tile([128, KH, VS], bf)
        nc.sync.dma_start(out=wfc_hi, in_=wfc_hi_d.ap().rearrange("(k p) v -> p k v", p=128))
        nc.sync.dma_start(out=wfc_lo, in_=wfc_lo_d.ap().rearrange("(k p) v -> p k v", p=128))

        # --------------- phase B: recurrence + interleaved logits ----------
        stage_tiles = [None] * MT    # [128, KH, 4, 64] bf16 per m-tile
        lsb_tiles = [None] * MT      # [128, VS] f32 logits rows
        filler = []                  # deferred logits/argmax jobs

        def logits_njob(m, n):
            def job():
                if n == 0:
                    lsb_tiles[m] = lsb_pool.tile([128, VS], f32,
                                                 name=f"lsb{m}", tag="lsb")
                ps = lps_pool.tile([128, 500], f32, name=f"lps{m}_{n}", tag="lps")
                st = stage_tiles[m]
                mms = []
                for k in range(KH):
                    hi = st[:, k, 0, :]
                    lo = st[:, k, 1, :]
                    wh = wfc_hi[:, k, n * 500:(n + 1) * 500]
                    wl = wfc_lo[:, k, n * 500:(n + 1) * 500]
                    mms.append((hi, wh))
                    mms.append((hi, wl))
                    mms.append((lo, wh))
                if fc_bias:
                    mms.append((ones1[:], bf_hi_sb[:, n * 500:(n + 1) * 500]))
                    mms.append((ones1[:], bf_lo_sb[:, n * 500:(n + 1) * 500]))
                for i, (lt, rh) in enumerate(mms):
                    nc.tensor.matmul(ps, lhsT=lt, rhs=rh,
                                     start=(i == 0), stop=(i == len(mms) - 1))
                nc.scalar.copy(lsb_tiles[m][:, n * 500:(n + 1) * 500], ps)
            return job

        def argmax_job(m):
            def job():
                lsb = lsb_tiles[m]
                mv = amax_pool.tile([128, 8], f32, name=f"mv{m}", tag="mv")
                mi = amax_pool.tile([128, 8], u32, name=f"mi{m}", tag="mi")
                mif = amax_pool.tile([128, 1], f32, name=f"mif{m}", tag="mif")
                nc.vector.max(out=mv, in_=lsb)
                nc.vector.max_index(out=mi, in_max=mv, in_values=lsb)
                nc.vector.tensor_copy(mif, mi[:, 0:1])
                rows = slice(m * 128, (m + 1) * 128)
                nc.sync.dma_start(out=logits_o[rows, :], in_=lsb)
                nc.sync.dma_start(out=maxv_o[rows, :], in_=mv[:, 0:1])
                nc.sync.dma_start(out=maxi_o[rows, :], in_=mif)
            return job

        c_prev = None
        for t in range(T):
            m, sp = t // 4, t % 4
            if sp == 0:
                stage_tiles[m] = stage_pool.tile([128, KH, 2, 128], bf,
                                                 name=f"stage{m}", tag="stage")
            # per-step Xg slice (prefetched)
            xg_t = xg_pool.tile([B, 2 * GS], bf, name=f"xg{t}", tag="xgt")
            nc.sync.dma_start(out=xg_t, in_=xg_dram[t * B:(t + 1) * B, :])

            gates = gps_pool.tile([B, GS], f32, name=f"g{t}", tag="gates")
            mms = []
            if t > 0:
                pm, psp = (t - 1) // 4, (t - 1) % 4
                pst = stage_tiles[pm]
                for k in range(KH):
                    hi = pst[:, k, 0, psp * 32:(psp + 1) * 32]
                    lo = pst[:, k, 1, psp * 32:(psp + 1) * 32]
                    mms.append((hi, whh_hi[:, k, :]))
                    mms.append((hi, whh_lo[:, k, :]))
                    mms.append((lo, whh_hi[:, k, :]))
            mms.append((ident32b[:], xg_t[:, 0:GS]))
            mms.append((ident32b[:], xg_t[:, GS:2 * GS]))
            for i, (lt, rh) in enumerate(mms):
                nc.tensor.matmul(gates, lhsT=lt, rhs=rh,
                                 start=(i == 0), stop=(i == len(mms) - 1))

            # activations: layout [i(0:128), f(128:256), o(256:384), g(384:512)]
            tng = small_pool.tile([B, 128], f32, name=f"tng{t}", tag="tng")
            nc.scalar.activation(tng, gates[:, 384:512], AF.Tanh)
            sif = small_pool.tile([B, 384], f32, name=f"sif{t}", tag="sif")
            nc.scalar.activation(sif, gates[:, 0:384], AF.Sigmoid)
            ig = small_pool.tile([B, 128], f32, name=f"ig{t}", tag="ig")
            nc.vector.tensor_mul(ig, sif[:, 0:128], tng)
            if t == 0:
                c_new = ig
            else:
                cf = small_pool.tile([B, 128], f32, name=f"cf{t}", tag="cf")
                nc.vector.tensor_mul(cf, sif[:, 128:256], c_prev)
                c_new = small_pool.tile([B, 128], f32, name=f"c{t}", tag="c")
                nc.vector.tensor_add(c_new, cf, ig)
            tc_t = small_pool.tile([B, 128], f32, name=f"tc{t}", tag="tc")
            nc.scalar.activation(tc_t, c_new, AF.Tanh)
            h_t = small_pool.tile([B, 128], f32, name=f"h{t}", tag="h")
            nc.vector.tensor_mul(h_t, sif[:, 256:384], tc_t)
            c_prev = c_new

            # transpose h -> [128, 32], split to bf16 hi/lo, all-gather
            trp = trp_pool.tile([128, 32], f32, name=f"trp{t}", tag="trp")
            nc.tensor.transpose(trp, h_t, ident32f)
            agp = agp_pool.tile([128, 64], bf, name=f"agp{t}", tag="agp")
            nc.scalar.copy(agp[:, 0:32], trp)
            nc.vector.tensor_sub(out=agp[:, 32:64], in0=trp, in1=agp[:, 0:32])
            agi = agd_pool.tile([128, 64], bf, name=f"agi{t}", tag="agi")
            nc.sync.dma_start(out=agi, in_=agp)
            ago = agd_pool.tile([1024, 64], bf, name=f"ago{t}", tag="ago",
                                addr_space="Shared")
            nc.gpsimd.collective_compute(
                "AllGather", mybir.AluOpType.bypass,
                replica_groups=[list(range(NC))],
                ins=[agi.opt()], outs=[ago.opt()],
            )
            ago_r = ago.rearrange("(k p) f -> p k f", p=128)
            nc.sync.dma_start(
                out=stage_tiles[m][:, :, 0, sp * 32:(sp + 1) * 32],
                in_=ago_r[:, :, 0:32])
            nc.sync.dma_start(
                out=stage_tiles[m][:, :, 1, sp * 32:(sp + 1) * 32],
                in_=ago_r[:, :, 32:64])

            # enqueue logits work for the m-tile completed at t-1
            if sp == 3 and m >= 1:
                mm_done = m - 1
                for n in range(NV):
                    filler.append(logits_njob(mm_done, n))
                filler.append(argmax_job(mm_done))
            # emit up to 2 deferred jobs per step
            for _ in range(2):
                if filler:
                    filler.pop(0)()

        # tail: last m-tile's logits + any remaining jobs
        for n in range(NV):
            filler.append(logits_njob(MT - 1, n))
        filler.append(argmax_job(MT - 1))
        while filler:
            filler.pop(0)()

        for p in (lsb_pool, wfc_pool, agd_pool, dram_pool, lps_pool, trp_pool,
                  gps_pool, amax_pool, agp_pool, small_pool, stage_pool,
                  xg_pool, whh_pool, cpool):
            p.release()

    nc.compile()
    return nc


def _get_program(gate_bias: bool, fc_bias: bool):
    key = (gate_bias, fc_bias)
    if key not in _PROG_CACHE:
        _PROG_CACHE[key] = _build_program(gate_bias, fc_bias)
    return _PROG_CACHE[key]


def _split_hi_lo(x):
    x = np.ascontiguousarray(x, dtype=np.float32)
    hi = x.astype(BF16)
    lo = (x - hi.astype(np.float32)).astype(BF16)
    return np.ascontiguousarray(hi), np.ascontiguousarray(lo)


def _prepare_in_maps(encoder, gtruths, emb, W_ih, W_hh, b_ih, b_hh, W_fc, b_fc):
    gate_bias = bool(np.any(b_ih) or np.any(b_hh))
    fc_bias = bool(np.any(b_fc))

    encT = np.ascontiguousarray(encoder.T)                  # [E, B]
    encT_hi, encT_lo = _split_hi_lo(encT)
    G = emb[gtruths]                                        # [B, T, D]
    G = np.transpose(G, (1, 0, 2)).reshape(T * B, D)        # row t*B+b
    GT_hi, GT_lo = _split_hi_lo(G.T)                        # [D, T*B]

    bias = b_ih + b_hh
    in_maps = []
    for k in range(NC):
        rows = np.concatenate([
            np.arange(k * 128, (k + 1) * 128) + g * H for g in (0, 1, 3, 2)
        ])  # [i, f, o, g] row order
        wk = W_ih[rows]                                     # [GS, E+D]
        wie_hi, wie_lo = _split_hi_lo(wk[:, :E].T)          # [E, GS]
        wid_hi, wid_lo = _split_hi_lo(wk[:, E:].T)          # [D, GS]
        whh_hi, whh_lo = _split_hi_lo(W_hh[rows].T)         # [H, GS]
        wfc_hi, wfc_lo = _split_hi_lo(W_fc[k * VS:(k + 1) * VS].T)  # [H, VS]
        m = {
            "encT_hi": encT_hi, "encT_lo": encT_lo,
            "GT_hi": GT_hi, "GT_lo": GT_lo,
            "wie_hi": wie_hi, "wie_lo": wie_lo,
            "wid_hi": wid_hi, "wid_lo": wid_lo,
            "whh_hi": whh_hi, "whh_lo": whh_lo,
            "wfc_hi": wfc_hi, "wfc_lo": wfc_lo,
        }
        if gate_bias:
            bh, bl = _split_hi_lo(bias[rows][None, :])
            m["bg_hi"] = bh; m["bg_lo"] = bl
        if fc_bias:
            bh, bl = _split_hi_lo(b_fc[k * VS:(k + 1) * VS][None, :])
            m["bf_hi"] = bh; m["bf_lo"] = bl
        in_maps.append(m)
    return in_maps, gate_bias, fc_bias


def _assemble(results):
    logits = np.concatenate([r["logits_o"] for r in results], axis=1)
    logits = logits.reshape(T, B, V)
    vals = np.stack([r["maxv_o"][:, 0] for r in results])   # [NC, T*B]
    idxs = np.stack([r["maxi_o"][:, 0] for r in results])   # [NC, T*B] (f32)
    best = np.argmax(vals, axis=0)                          # first max wins
    flat = best * VS + idxs[best, np.arange(T * B)].astype(np.int64)
    predic = flat.reshape(T, B).T.astype(np.int32)
    return logits, predic


def kernel(encoder, gtruths, emb, W_ih, W_hh, b_ih, b_hh, W_fc, b_fc,
           ssprob=1, is_train=1, **_ignored):
    encoder = np.asarray(encoder, dtype=np.float32)
    gtruths = np.asarray(gtruths)
    emb = np.asarray(emb, dtype=np.float32)
    W_ih = np.asarray(W_ih, dtype=np.float32)
    W_hh = np.asarray(W_hh, dtype=np.float32)
    b_ih = np.asarray(b_ih, dtype=np.float32)
    b_hh = np.asarray(b_hh, dtype=np.float32)
    W_fc = np.asarray(W_fc, dtype=np.float32)
    b_fc = np.asarray(b_fc, dtype=np.float32)

    in_maps, gate_bias, fc_bias = _prepare_in_maps(
        encoder, gtruths, emb, W_ih, W_hh, b_ih, b_hh, W_fc, b_fc)
    nc = _get_program(gate_bias, fc_bias)
    res = bass_utils.run_bass_kernel_spmd(nc, in_maps, core_ids=list(range(NC)))
    return _assemble(res.results)
